# revision 9
# baseline (speedup 1.0000x reference)
"""Distributed Bass kernel for GQA causal attention (B=2, S=2048, H=2048,
NH=16, NKV=4, HD=128) on 8 TRN2 NeuronCores.

Sharding: core c (0..7) handles batch b = c//4 and kv-group g = c%4
(4 query heads + 1 kv head, GQA groups kept intact).  wq/wk/wv are
column-sharded, wo row-sharded; each core emits a partial output
[H, S] (transposed) and the host sums the 4 group-partials per batch.

Layout strategy on device (bf16 matmul inputs, f32 PSUM accumulation):
  - x is fed pre-transposed (xT[h, s]) so QKV projections produce
    Q^T/K^T/V^T in [d, s] layout directly (d=128 = one partition tile).
  - RoPE rotate_half is a constant 128x128 matmul (R^T as lhsT);
    cos/sin are fed pre-transposed, kept f32.
  - scores are computed transposed: ST[kj, qi] = K^T(kj)·Q(qi), so
    softmax needs no on-chip transposes.  Two kj-tiles of scores share
    one [128, 1024] PSUM pair-tile (2 banks) so ONE Exp activation
    covers both (halves ACT instruction overhead).  exp via ACT with
    the 1/sqrt(HD) scale folded; no max-subtraction (scores are O(1)).
  - causal mask = additive -1e30 on the PSUM scores of the diagonal
    band only; fully-masked tiles are never computed.
  - the kj pair loop is software-pipelined: scores for pair p+1 issue
    before attnV/rowsum of pair p, hiding the Exp latency.
  - rowsums are accumulated PRE-BROADCAST: lhsT = ones[128,128] gives
    a [128, SB] PSUM tile whose every row is the rowsum (same PE cost
    as a [1, SB] rowsum, but no separate broadcast matmul and no PE
    dependency on the reciprocal).  Normalization is then DVE-only:
    reciprocal_approx_fast + tensor_mul, emitted one att-block late so
    the PE never waits on it.
  - phase 3 (wo projection) is interleaved per q-block (qb-outer loop)
    so output DMA drains throughout phase 2 instead of in a tail.
    o_ps pair-tiles share the scores PSUM pool (2-bank slots).
"""

import math
import os
import sys

import ml_dtypes
import numpy as np

sys.path.insert(0, "/opt/trn_rl_repo")

import concourse.bass as bass
import concourse.mybir as mybir
import concourse.tile as tile
from concourse.bass_utils import run_bass_kernel_spmd

B, S, H = 2, 2048, 2048
NH, NKV, HD = 16, 4, 128
NREP = NH // NKV
NCORES = 8
GH = 4                # q-heads per core (one kv group)
P = 128
SB = 512              # s-block width (matmul moving free dim)
NB = S // SB          # 4 s-blocks
NT = S // P           # 16 partition tiles along s / h / e
SCALE = 1.0 / math.sqrt(HD)
F32 = mybir.dt.float32
F32R = mybir.dt.float32r
BF16 = mybir.dt.bfloat16
MM_MODE = os.environ.get("BASS_MM_DTYPE", "bf16")  # bf16 | f32r | f32
USE_F32R = MM_MODE == "f32r"
MMDT = {"bf16": BF16, "f32r": F32R, "f32": F32}[MM_MODE]
NPMM = ml_dtypes.bfloat16 if MM_MODE == "bf16" else np.float32
OUT_BF16 = os.environ.get("BASS_OUT_BF16", "1") == "1" and MM_MODE == "bf16"
OUTDT = BF16 if OUT_BF16 else F32
NPOUT = ml_dtypes.bfloat16 if OUT_BF16 else np.float32


def _consts():
    npdt = NPMM
    # rotate_half as matmul: rot = RT.T @ q  (RT is the lhsT)
    RT = np.zeros((P, P), npdt)
    idx = np.arange(64)
    RT[idx + 64, idx] = -1.0
    RT[idx, idx + 64] = 1.0
    # canonical causal additive triangle: 0 iff kj_local <= qi_local
    kjl = np.arange(P)[:, None]
    qil = np.arange(P)[None, :]
    masks = np.where(kjl <= qil, 0.0, -1e30).astype(np.float32)
    ident = np.eye(P, dtype=npdt)
    ones_f = np.ones((P, P), npdt)
    return RT, masks, ident, ones_f


def build_nc():
    nc = bass.Bass()

    xT_d = nc.declare_dram_parameter("xT", [H, S], MMDT, isOutput=False)
    wq_d = nc.declare_dram_parameter("wq", [H, GH * HD], MMDT, isOutput=False)
    wk_d = nc.declare_dram_parameter("wk", [H, HD], MMDT, isOutput=False)
    wv_d = nc.declare_dram_parameter("wv", [H, HD], MMDT, isOutput=False)
    wo_d = nc.declare_dram_parameter("wo", [GH * HD, H], MMDT, isOutput=False)
    cosT_d = nc.declare_dram_parameter("cosT", [HD, S], F32, isOutput=False)
    sinT_d = nc.declare_dram_parameter("sinT", [HD, S], F32, isOutput=False)
    out_d = nc.declare_dram_parameter("out", [H, S], OUTDT, isOutput=True)

    RT_np, masks_np, ident_np, ones_f_np = _consts()
    RT_d = nc.inline_tensor(RT_np, "rot_t")
    masks_d = nc.inline_tensor(masks_np, "masks")
    ident_d = nc.inline_tensor(ident_np, "ident")
    ones_f_d = nc.inline_tensor(ones_f_np, "ones_f")

    def _mr(ap):
        """matmul-feeding const: reinterpret f32-typed DRAM as f32r only
        in f32r mode; bf16 consts are created in bf16 directly."""
        return ap.bitcast(F32R) if USE_F32R else ap

    with tile.TileContext(nc) as tc, \
         tc.tile_pool(name="persist", bufs=1) as persist:
        rt_sb = persist.tile([P, P], MMDT, tag="rt")
        masks_sb = persist.tile([P, P], F32, tag="masks")
        ident_sb = persist.tile([P, P], MMDT, tag="ident")
        ones_sb = persist.tile([P, P], MMDT, tag="ones_f")
        cos_sb = persist.tile([P, S], F32, tag="cos")
        sin_sb = persist.tile([P, S], F32, tag="sin")

        # resident weights (each element used once per s-block)
        wq_sb = persist.tile([P, NT, GH * HD], MMDT, tag="wq")
        wk_sb = persist.tile([P, NT, HD], MMDT, tag="wk")
        wv_sb = persist.tile([P, NT, HD], MMDT, tag="wv")

        # per-(head, s-block) roped Q; per-s-block K^T, V^T, V tiles.
        # Block granularity keeps cross-phase dependencies fine-grained.
        QR = [[persist.tile([P, SB], MMDT, tag=f"qr{h}_{b}", name=f"qr{h}_{b}")
               for b in range(NB)] for h in range(GH)]
        KR = [persist.tile([P, SB], MMDT, tag=f"kr{b}", name=f"kr{b}")
              for b in range(NB)]
        VT = [persist.tile([P, SB], MMDT, tag=f"vt{b}", name=f"vt{b}")
              for b in range(NB)]
        VV = [persist.tile([P, SB], MMDT, tag=f"vv{b}", name=f"vv{b}")
              for b in range(NB)]

        def kr_t(kj):
            return KR[kj // 4][:, (kj % 4) * P:(kj % 4 + 1) * P]

        def vv_t(kj):
            return VV[kj // 4][:, (kj % 4) * P:(kj % 4 + 1) * P]

        # normalized attention outputs, per (head, q-block)
        OT = [[persist.tile([P, SB], MMDT, tag=f"ot{h}_{b}", name=f"ot{h}_{b}")
               for b in range(NB)] for h in range(GH)]

        # ---------------- Phase 1: projections + RoPE ----------------
        with (
            tc.tile_pool(name="xp", bufs=24) as xp,
            tc.tile_pool(name="p1w", bufs=3) as p1w,
            tc.tile_pool(name="p1ps", bufs=1, space="PSUM") as p1ps,
            tc.tile_pool(name="rotps", bufs=1, space="PSUM") as rotps,
        ):
            # critical-path DMAs first, interleaved per-t so the PE can
            # start the t-loop as soon as tile 0 lands
            xts0 = []
            for t in range(NT):
                xtt = xp.tile([P, SB], MMDT, tag="x", name=f"x0_{t}")
                nc.sync.dma_start(out=xtt, in_=xT_d[t * P:(t + 1) * P, 0:SB])
                nc.sync.dma_start(out=wq_sb[:, t, :],
                                  in_=wq_d[t * P:(t + 1) * P, :])
                nc.sync.dma_start(out=wk_sb[:, t, :],
                                  in_=wk_d[t * P:(t + 1) * P, :])
                nc.sync.dma_start(out=wv_sb[:, t, :],
                                  in_=wv_d[t * P:(t + 1) * P, :])
                xts0.append(xtt)
            xts1 = []
            for t in range(8):
                xtt = xp.tile([P, SB], MMDT, tag="x", name=f"x1_{t}")
                nc.sync.dma_start(out=xtt, in_=xT_d[t * P:(t + 1) * P,
                                                    SB:2 * SB])
                xts1.append(xtt)
            nc.sync.dma_start(out=rt_sb, in_=_mr(RT_d[:]))
            nc.sync.dma_start(out=ident_sb, in_=_mr(ident_d[:]))
            nc.sync.dma_start(out=ones_sb, in_=_mr(ones_f_d[:]))
            nc.sync.dma_start(out=cos_sb, in_=cosT_d[:])
            nc.sync.dma_start(out=sin_sb, in_=sinT_d[:])
            nc.sync.dma_start(out=masks_sb, in_=masks_d[:])
            for sb in range(NB):
                ssl = slice(sb * SB, (sb + 1) * SB)
                if sb == 0:
                    xt = xts0
                else:
                    xt = list(xts1) if sb == 1 else []
                    for t in range(len(xt), NT):
                        xtt = xp.tile([P, SB], MMDT, tag="x",
                                      name=f"x{sb}_{t}")
                        nc.sync.dma_start(out=xtt,
                                          in_=xT_d[t * P:(t + 1) * P, ssl])
                        xt.append(xtt)
                ps = [p1ps.tile([P, SB], F32, tag=f"ps{i}", name=f"ps{i}")
                      for i in range(6)]
                for t in range(NT):
                    st, sp = (t == 0), (t == NT - 1)
                    for h in range(GH):
                        nc.tensor.matmul(
                            ps[h], wq_sb[:, t, h * HD:(h + 1) * HD],
                            xt[t], start=st, stop=sp,
                        )
                    nc.tensor.matmul(ps[4], wk_sb[:, t, :], xt[t],
                                     start=st, stop=sp)
                    nc.tensor.matmul(ps[5], wv_sb[:, t, :], xt[t],
                                     start=st, stop=sp)
                # RoPE for Q heads and K; V^T plain copy.
                for i in range(5):
                    raw_r = p1w.tile([P, SB], MMDT, tag="raw",
                                     name=f"raw{sb}_{i}")
                    nc.vector.tensor_copy(raw_r, ps[i])
                    rot = rotps.tile([P, SB], F32, tag="rot",
                                     name=f"rot{sb}_{i}")
                    nc.tensor.matmul(rot, rt_sb, raw_r)
                    t1 = p1w.tile([P, SB], F32, tag="t1", name=f"t1_{sb}_{i}")
                    nc.vector.tensor_mul(t1, raw_r, cos_sb[:, ssl])
                    t2 = p1w.tile([P, SB], F32, tag="t2", name=f"t2_{sb}_{i}")
                    nc.vector.tensor_mul(t2, rot, sin_sb[:, ssl])
                    dst = QR[i][sb] if i < GH else KR[sb]
                    nc.vector.tensor_add(dst, t1, t2)
                nc.vector.tensor_copy(VT[sb], ps[5])
                # transpose this block's V^T -> V[s, d] tiles right away
                for tt in range(SB // P):
                    vps = rotps.tile([P, P], MMDT, tag="vtr",
                                     name=f"vtr{sb}_{tt}")
                    nc.tensor.transpose(vps, VT[sb][:, tt * P:(tt + 1) * P],
                                        ident_sb)
                    nc.vector.tensor_copy(VV[sb][:, tt * P:(tt + 1) * P], vps)

        # -------- Phase 2+3: attention (qb outer) + wo projection --------
        with (
            tc.tile_pool(name="p2w", bufs=5) as p2w,
            tc.tile_pool(name="recp", bufs=3) as recp,
            tc.tile_pool(name="oep", bufs=4) as oep,
            tc.tile_pool(name="pairps", bufs=2, space="PSUM") as pairps,
            tc.tile_pool(name="accps", bufs=4, space="PSUM") as accps,
        ):
            # wo shares wq_sb's slot (dead after phase 1); prefetch during
            # attention so the wo projection starts without a DMA stall
            wo_sb = wq_sb.rearrange("p a b -> p (a b)").rearrange(
                "p (g e) -> p g e", g=GH)
            for hh in range(GH):
                nc.sync.dma_start(out=wo_sb[:, hh, :],
                                  in_=wo_d[hh * P:(hh + 1) * P, :])

            pending_av = None    # deferred attnV/rowsum of the previous pair
            pending_norm = []    # deferred DVE normalizations

            def flush_av():
                nonlocal pending_av
                if pending_av is not None:
                    pending_av()
                    pending_av = None

            def flush_norms():
                while pending_norm:
                    pending_norm.pop(0)()

            for qb in range(NB):
                qsl = slice(qb * SB, (qb + 1) * SB)
                nkj = 4 * (qb + 1)
                for h in range(GH):
                    ot_ps = accps.tile([P, SB], F32, tag="acc",
                                       name=f"otp{h}_{qb}")
                    rs_ps = accps.tile([P, SB], F32, tag="acc",
                                       name=f"rsp{h}_{qb}")
                    for pi in range(nkj // 2):
                        kjs = (2 * pi, 2 * pi + 1)
                        st = pairps.tile([P, 2 * SB], F32, tag="pair",
                                         name=f"st{h}_{qb}_{pi}")
                        offs = []
                        for idx, kj in enumerate(kjs):
                            j = kj - (nkj - 4)
                            q0 = 0 if j < 0 else P * j
                            W = SB - q0
                            off = 0 if idx == 0 else SB
                            offs.append((kj, q0, W, off))
                            nc.tensor.matmul(
                                st[:, off:off + W], kr_t(kj),
                                QR[h][qb][:, q0:], start=True, stop=True,
                                skip_group_check=True)
                        for kj, q0, W, off in offs:
                            if kj - (nkj - 4) >= 0:
                                nc.vector.tensor_add(
                                    st[:, off:off + P], st[:, off:off + P],
                                    masks_sb)
                        wid = offs[-1][3] + offs[-1][2]
                        p_sb = p2w.tile([P, 2 * SB], MMDT, tag="p",
                                        name=f"p{h}_{qb}_{pi}")
                        nc.scalar.activation(
                            p_sb[:, :wid], st[:, :wid],
                            mybir.ActivationFunctionType.Exp, scale=SCALE)
                        flush_av()

                        def _av(offs=offs, p_sb=p_sb, ot_ps=ot_ps,
                                rs_ps=rs_ps, nkj=nkj):
                            for kj, q0, W, off in offs:
                                first, last = (kj == 0), (kj == nkj - 1)
                                nc.tensor.matmul(
                                    ot_ps[:, q0:], vv_t(kj),
                                    p_sb[:, off:off + W],
                                    start=first, stop=last,
                                    skip_group_check=True)
                                nc.tensor.matmul(
                                    rs_ps[:, q0:], ones_sb,
                                    p_sb[:, off:off + W],
                                    start=first, stop=last,
                                    skip_group_check=True)
                        pending_av = _av
                        if pi == 1:
                            flush_norms()

                    def _norm(h=h, qb=qb, ot_ps=ot_ps, rs_ps=rs_ps):
                        # 1/rs = exp(-ln(rs)) on ACT: Ln/Exp share one
                        # activation table set, and ACT has slack while
                        # DVE reciprocal would cost ~6.5ns/elem.
                        lnr = recp.tile([P, SB], F32, tag="lnr",
                                        name=f"lnr{h}_{qb}")
                        nc.scalar.activation(
                            lnr, rs_ps, mybir.ActivationFunctionType.Ln)
                        rec = recp.tile([P, SB], F32, tag="rec",
                                        name=f"rec{h}_{qb}")
                        nc.scalar.activation(
                            rec, lnr, mybir.ActivationFunctionType.Exp,
                            scale=-1.0)
                        nc.vector.tensor_mul(OT[h][qb], ot_ps, rec)
                    pending_norm.append(_norm)

                # ---- wo projection for this q-block ----
                flush_av()
                flush_norms()
                for ep in range(NT // 2):
                    o_ps = pairps.tile([P, 2 * SB], F32, tag="pair",
                                       name=f"wop{qb}_{ep}")
                    for half in range(2):
                        e = 2 * ep + half
                        for hh in range(GH):
                            nc.tensor.matmul(
                                o_ps[:, half * SB:(half + 1) * SB],
                                wo_sb[:, hh, e * P:(e + 1) * P],
                                OT[hh][qb],
                                start=(hh == 0), stop=(hh == GH - 1),
                                skip_group_check=True)
                    oe = oep.tile([P, 2 * SB], OUTDT, tag="oe",
                                  name=f"oe{qb}_{ep}")
                    nc.vector.tensor_copy(oe, o_ps)
                    for half in range(2):
                        e = 2 * ep + half
                        nc.sync.dma_start(
                            out=out_d[e * P:(e + 1) * P, qsl],
                            in_=oe[:, half * SB:(half + 1) * SB])

    _hoist_matmul_waits(nc)
    return nc


_HOIST_OPS = {"Matmult", "DMACopy"}


def _hoist_matmul_waits(nc):
    """Self-loading f32r matmuls (and direct2d DMAs) only support ONE
    sync-wait — walrus puts all waits on one ISA struct.  Hoist extra
    waits onto standalone single-wait EventSemaphores inserted right
    before the offending instruction on the same engine."""
    n_fixed = 0
    for fn in nc.m.functions:
        for blk in fn.blocks:
            out = []
            for inst in blk.instructions:
                si = inst.sync_info
                if (inst.opcode != "EventSemaphore" and si is not None
                        and si.on_wait is not None and len(si.on_wait) > 1):
                    waits = list(si.on_wait)
                    for wi, w in enumerate(waits[:-1]):
                        out.append(mybir.InstEventSemaphore(
                            name=f"hoistw_{inst.name}_{wi}", ins=[], outs=[],
                            sync_info=mybir.SyncInfo(on_wait=[w],
                                                     on_update=[]),
                            engine=inst.engine))
                    inst.sync_info = mybir.SyncInfo(
                        on_wait=[waits[-1]],
                        on_update=list(si.on_update or []))
                    n_fixed += 1
                out.append(inst)
            blk.instructions = out
    return n_fixed


def make_in_maps(x, cos, sin, wq, wk, wv, wo):
    cosT = np.ascontiguousarray(cos.T.astype(np.float32))
    sinT = np.ascontiguousarray(sin.T.astype(np.float32))
    xT = [np.ascontiguousarray(x[b].T.astype(NPMM)) for b in range(B)]
    wq, wk, wv, wo = (a.astype(NPMM) for a in (wq, wk, wv, wo))
    in_maps = []
    for c in range(NCORES):
        b, g = divmod(c, NKV)
        in_maps.append({
            "xT": xT[b],
            "wq": np.ascontiguousarray(wq[:, g * GH * HD:(g + 1) * GH * HD]),
            "wk": np.ascontiguousarray(wk[:, g * HD:(g + 1) * HD]),
            "wv": np.ascontiguousarray(wv[:, g * HD:(g + 1) * HD]),
            "wo": np.ascontiguousarray(wo[g * GH * HD:(g + 1) * GH * HD, :]),
            "cosT": cosT,
            "sinT": sinT,
        })
    return in_maps


_NC_CACHE = {}


def _get_nc():
    if "nc" not in _NC_CACHE:
        _NC_CACHE["nc"] = build_nc()
    return _NC_CACHE["nc"]


def run(x, cos, sin, wq, wk, wv, wo, **spmd_kwargs):
    nc = _get_nc()
    in_maps = make_in_maps(x, cos, sin, wq, wk, wv, wo)
    res = run_bass_kernel_spmd(nc, in_maps, core_ids=list(range(NCORES)),
                               **spmd_kwargs)
    outs = [np.asarray(res.results[c]["out"]).astype(np.float32)
            for c in range(NCORES)]
    full = np.empty((B, S, H), np.float32)
    for b in range(B):
        acc = outs[4 * b]
        for g in range(1, NKV):
            acc = acc + outs[4 * b + g]
        full[b] = acc.T
    return full, res


def kernel(**inputs):
    out, _ = run(**inputs)
    return out


if __name__ == "__main__":
    import tempfile
    from concourse.bass_utils import compile_bir_kernel

    nc = build_nc()
    print("graph built OK")
    if os.environ.get("COMPILE_CHECK", "1") == "1":
        td = tempfile.mkdtemp(prefix="bass_compile_")
        neff = compile_bir_kernel(nc.to_json_bytes(), td, "kernel.neff")
        print(f"compiled OK: {neff}")
